# revision 10
# baseline (speedup 1.0000x reference)
"""MixLinear GEMM kernel for 8 Trainium2 NeuronCores.

Computation (per reference):
  y = (q_x @ W^T) * x_scale * scale_col + act_outliers @ Wc^T + bias
where q_x is the per-row int8 quantization of x with outlier columns
(ind) extracted and zeroed.

Sharding: column-parallel — W, scale_col, weight_cache, bias sharded on
OUT (28672 -> 8 x 3584); x / ind replicated; outputs concatenated on OUT.

Device strategy (per core):
  - Weight shard [3584, 8192] int8 is viewed as uint16 [3584, 4096] and
    DMA-transposed (X-bar, 2-byte granularity) straight from DRAM into
    SBUF tiles wt[p, t, j] holding int8 PAIRS (IN indices 256t+2p,
    256t+2p+1) for OUT column j.
  - The pairs are de-interleaved with byte-strided cast copies
    (int8 -> fp16) split across DVE / ACT / GPSIMD.
  - 64 fp16 matmuls per 512-wide OUT chunk accumulate q_x^T-stationary
    products into PSUM [32, 512] fp32; activation tiles xtp are
    pre-permuted to match the even/odd interleaved K order.
  - Outlier GEMM (aoT stationary, wcT moving) into a second PSUM bank;
    epilogue applies x_scale (per-row), scale_col (per-col), adds
    outlier GEMM + bias, and stores fp16.
"""
import sys
import time

sys.path.insert(0, '/opt/trn_rl_repo')

from contextlib import ExitStack

import numpy as np

import concourse.bass as bass
import concourse.tile as tile
from concourse import bacc
from concourse import mybir
from concourse.bass_utils import run_bass_kernel_spmd

N_CORES = 8
IN, OUT, FP, M = 8192, 28672, 128, 32
OUT_SH = OUT // N_CORES          # 3584 out rows per core
OC = 512                         # out-chunk width
N_OC = OUT_SH // OC              # 7
NT = IN // 256                   # 32 pair-tiles of 256 IN
NS = 2 * NT                      # 64 K-slices of 128

LAST_RESULTS = None


def _build_nc():
    nc = bacc.Bacc("TRN2", target_bir_lowering=False, debug=False)

    w16 = nc.declare_dram_parameter("w16", [OUT_SH, IN // 2], mybir.dt.uint16, isOutput=False)
    xtp = nc.declare_dram_parameter("xtp", [128, NS, M], mybir.dt.float16, isOutput=False)
    aot = nc.declare_dram_parameter("aot", [FP, M], mybir.dt.float16, isOutput=False)
    wct = nc.declare_dram_parameter("wct", [FP, OUT_SH], mybir.dt.float16, isOutput=False)
    xs = nc.declare_dram_parameter("xs", [M, 1], mybir.dt.float32, isOutput=False)
    sc = nc.declare_dram_parameter("sc", [M, OUT_SH], mybir.dt.float16, isOutput=False)
    bs = nc.declare_dram_parameter("bs", [M, OUT_SH], mybir.dt.float16, isOutput=False)
    y = nc.declare_dram_parameter("y", [M, OUT_SH], mybir.dt.float16, isOutput=True)

    with tile.TileContext(nc) as tc, ExitStack() as ctx:
        const = ctx.enter_context(tc.tile_pool(name="const", bufs=1))
        wpool = ctx.enter_context(tc.tile_pool(name="w", bufs=2))
        eopool = ctx.enter_context(tc.tile_pool(name="eo", bufs=8))
        epool = ctx.enter_context(tc.tile_pool(name="ep", bufs=2))
        pmain = ctx.enter_context(tc.tile_pool(name="pm", bufs=2, space="PSUM"))
        pout = ctx.enter_context(tc.tile_pool(name="po", bufs=2, space="PSUM"))

        xtp_sb = const.tile([128, NS, M], mybir.dt.float16)
        nc.sync.dma_start(out=xtp_sb[:], in_=xtp.ap())
        aot_sb = const.tile([FP, M], mybir.dt.float16)
        nc.sync.dma_start(out=aot_sb[:], in_=aot.ap())
        wct_sb = const.tile([FP, OUT_SH], mybir.dt.float16)
        nc.sync.dma_start(out=wct_sb[:], in_=wct.ap())
        xs_sb = const.tile([M, 1], mybir.dt.float32)
        nc.sync.dma_start(out=xs_sb[:], in_=xs.ap())
        sc_sb = const.tile([M, OUT_SH], mybir.dt.float16)
        nc.sync.dma_start(out=sc_sb[:], in_=sc.ap())
        bs_sb = const.tile([M, OUT_SH], mybir.dt.float16)
        nc.sync.dma_start(out=bs_sb[:], in_=bs.ap())

        engines = [nc.vector, nc.scalar]

        for oc in range(N_OC):
            wt = wpool.tile([128, NT, OC], mybir.dt.uint16, tag="wt")
            nc.sync.dma_start(
                out=wt[:], in_=w16.ap()[oc * OC:(oc + 1) * OC, :],
                transpose=True)
            wt_b = wt[:].bitcast(mybir.dt.int8).rearrange(
                "p t (j e) -> p t j e", e=2)

            y_ps = pmain.tile([M, OC], mybir.dt.float32, tag="ymm")
            for t in range(NT):
                for e in (0, 1):
                    s = 2 * t + e
                    eo = eopool.tile([128, OC], mybir.dt.float16, tag="eo")
                    eng = engines[s % 2]
                    if eng is nc.scalar:
                        eng.copy(eo[:], wt_b[:, t, :, e])
                    else:
                        eng.tensor_copy(eo[:], wt_b[:, t, :, e])
                    nc.tensor.matmul(
                        y_ps[:], xtp_sb[:, s, :], eo[:],
                        start=(s == 0), stop=(s == NS - 1))

            o_ps = pout.tile([M, OC], mybir.dt.float32, tag="omm")
            nc.tensor.matmul(
                o_ps[:], aot_sb[:], wct_sb[:, oc * OC:(oc + 1) * OC],
                start=True, stop=True)

            # epilogue mirrors the reference's fp16 chain exactly,
            # including the overflowing int32->fp16 cast of y_int
            # (values > 65504 must become +/-inf):
            #   y16 = fp16(y_int); t1 = y16*xs; t2 = t1*sc;
            #   t3 = t2 + fp16(outlierGEMM); y = t3 + bias
            y16 = epool.tile([M, OC], mybir.dt.float16, tag="y16")
            nc.vector.tensor_copy(y16[:], y_ps[:])
            o16 = epool.tile([M, OC], mybir.dt.float16, tag="o16")
            nc.vector.tensor_copy(o16[:], o_ps[:])
            t1 = epool.tile([M, OC], mybir.dt.float16, tag="t1")
            nc.vector.tensor_scalar_mul(t1[:], y16[:], xs_sb[:])
            t2 = epool.tile([M, OC], mybir.dt.float16, tag="t2")
            nc.vector.tensor_mul(t2[:], t1[:], sc_sb[:, oc * OC:(oc + 1) * OC])
            t3 = epool.tile([M, OC], mybir.dt.float16, tag="t3")
            nc.vector.tensor_add(t3[:], t2[:], o16[:])
            yo = epool.tile([M, OC], mybir.dt.float16, tag="yo")
            nc.vector.tensor_add(yo[:], t3[:], bs_sb[:, oc * OC:(oc + 1) * OC])
            nc.sync.dma_start(out=y.ap()[:, oc * OC:(oc + 1) * OC], in_=yo[:])

    nc.compile()
    return nc


_NC_CACHE = None


def kernel(x, weight, scale_col, weight_cache, bias, ind):
    global LAST_RESULTS, _NC_CACHE

    x2 = np.asarray(x, dtype=np.float16).reshape(M, IN)
    weight = np.asarray(weight, dtype=np.int8)
    scale_col = np.asarray(scale_col, dtype=np.float16).reshape(OUT)
    weight_cache = np.asarray(weight_cache, dtype=np.float16)
    bias = np.asarray(bias, dtype=np.float16).reshape(OUT)
    ind = np.asarray(ind, dtype=np.int32)

    # ---- x-side prep (fp16 semantics to match reference) ----
    act_outliers = x2[:, ind]                              # [M, FP]
    tmp = x2.copy()
    tmp[:, ind] = np.float16(0)
    x_scale = np.max(np.abs(tmp), axis=1, keepdims=True) / np.float16(127)
    q = np.clip(np.round(tmp / x_scale), -128, 127).astype(np.float16)

    # xtp[k, 2t+e, m] = q[m, 256t + 2k + e]
    arr = q.reshape(M, NT, 128, 2)                         # [m, t, k, e]
    xtp = np.ascontiguousarray(arr.transpose(2, 1, 3, 0).reshape(128, NS, M))
    aot = np.ascontiguousarray(act_outliers.T)             # [FP, M]
    xs = x_scale.astype(np.float16).astype(np.float32)     # [M, 1], fp16-valued

    if _NC_CACHE is None:
        _NC_CACHE = _build_nc()
    nc = _NC_CACHE

    in_maps = []
    for c in range(N_CORES):
        lo, hi = c * OUT_SH, (c + 1) * OUT_SH
        w_sh = np.ascontiguousarray(weight[lo:hi]).view(np.uint16)
        in_maps.append({
            "w16": w_sh,
            "xtp": xtp,
            "aot": aot,
            "wct": np.ascontiguousarray(weight_cache[lo:hi].T),
            "xs": xs,
            "sc": np.ascontiguousarray(
                np.broadcast_to(scale_col[lo:hi], (M, OUT_SH))),
            "bs": np.ascontiguousarray(
                np.broadcast_to(bias[lo:hi], (M, OUT_SH))),
        })

    last_err = None
    for attempt in range(3):
        try:
            LAST_RESULTS = run_bass_kernel_spmd(
                nc, in_maps, list(range(N_CORES)))
            break
        except Exception as err:  # transient NRT exec-unit errors recover on retry
            last_err = err
            print(f"kernel: run attempt {attempt} failed ({type(err).__name__}); retrying",
                  file=sys.stderr)
            time.sleep(2.0)
    else:
        raise last_err
    parts = [LAST_RESULTS.results[c]["y"] for c in range(N_CORES)]
    out = np.concatenate(parts, axis=1).reshape(M, 1, OUT)
    return out.astype(np.float16)


# revision 13
# speedup vs baseline: 1.0138x; 1.0138x over previous
"""MixLinear GEMM kernel for 8 Trainium2 NeuronCores.

Computation (per reference):
  y = (q_x @ W^T) * x_scale * scale_col + act_outliers @ Wc^T + bias
where q_x is the per-row int8 quantization of x with outlier columns
(ind) extracted and zeroed.

Sharding: column-parallel — W, scale_col, weight_cache, bias sharded on
OUT (28672 -> 8 x 3584); x / ind replicated; outputs concatenated on OUT.

Device strategy (per core):
  - Weight shard [3584, 8192] int8 is viewed as uint16 [3584, 4096] and
    DMA-transposed (X-bar, 2-byte granularity) straight from DRAM into
    SBUF tiles wt[p, t, j] holding int8 PAIRS (IN indices 256t+2p,
    256t+2p+1) for OUT column j.
  - The pairs are de-interleaved with byte-strided cast copies
    (int8 -> fp16) split across DVE / ACT / GPSIMD.
  - 64 fp16 matmuls per 512-wide OUT chunk accumulate q_x^T-stationary
    products into PSUM [32, 512] fp32; activation tiles xtp are
    pre-permuted to match the even/odd interleaved K order.
  - Outlier GEMM (aoT stationary, wcT moving) into a second PSUM bank;
    epilogue applies x_scale (per-row), scale_col (per-col), adds
    outlier GEMM + bias, and stores fp16.
"""
import sys
import time

sys.path.insert(0, '/opt/trn_rl_repo')

from contextlib import ExitStack

import numpy as np

import concourse.bass as bass
import concourse.tile as tile
from concourse import bacc
from concourse import mybir
from concourse.bass_utils import run_bass_kernel_spmd

N_CORES = 8
IN, OUT, FP, M = 8192, 28672, 128, 32
OUT_SH = OUT // N_CORES          # 3584 out rows per core
OC = 512                         # out-chunk width
N_OC = OUT_SH // OC              # 7
NT = IN // 256                   # 32 pair-tiles of 256 IN
NS = 2 * NT                      # 64 K-slices of 128

LAST_RESULTS = None


def _build_nc():
    nc = bacc.Bacc("TRN2", target_bir_lowering=False, debug=False)

    w16 = nc.declare_dram_parameter("w16", [OUT_SH, IN // 2], mybir.dt.uint16, isOutput=False)
    xtp = nc.declare_dram_parameter("xtp", [128, NS, M], mybir.dt.float16, isOutput=False)
    aot = nc.declare_dram_parameter("aot", [FP, M], mybir.dt.float16, isOutput=False)
    wct = nc.declare_dram_parameter("wct", [FP, OUT_SH], mybir.dt.float16, isOutput=False)
    xs = nc.declare_dram_parameter("xs", [M, 1], mybir.dt.float32, isOutput=False)
    sc = nc.declare_dram_parameter("sc", [M, OUT_SH], mybir.dt.float16, isOutput=False)
    bs = nc.declare_dram_parameter("bs", [M, OUT_SH], mybir.dt.float16, isOutput=False)
    y = nc.declare_dram_parameter("y", [M, OUT_SH], mybir.dt.float16, isOutput=True)

    with tile.TileContext(nc) as tc, ExitStack() as ctx:
        const = ctx.enter_context(tc.tile_pool(name="const", bufs=1))
        wpool = ctx.enter_context(tc.tile_pool(name="w", bufs=2))
        eopool = ctx.enter_context(tc.tile_pool(name="eo", bufs=6))
        epool = ctx.enter_context(tc.tile_pool(name="ep", bufs=2))
        pmain = ctx.enter_context(tc.tile_pool(name="pm", bufs=2, space="PSUM"))
        pout = ctx.enter_context(tc.tile_pool(name="po", bufs=2, space="PSUM"))

        xtp_sb = const.tile([128, NS, M], mybir.dt.float16)
        nc.sync.dma_start(out=xtp_sb[:], in_=xtp.ap())
        aot_sb = const.tile([FP, M], mybir.dt.float16)
        nc.sync.dma_start(out=aot_sb[:], in_=aot.ap())
        wct_sb = const.tile([FP, OUT_SH], mybir.dt.float16)
        nc.sync.dma_start(out=wct_sb[:], in_=wct.ap())
        xs_sb = const.tile([M, 1], mybir.dt.float32)
        nc.sync.dma_start(out=xs_sb[:], in_=xs.ap())
        sc_sb = const.tile([M, OUT_SH], mybir.dt.float16)
        nc.sync.dma_start(out=sc_sb[:], in_=sc.ap())
        bs_sb = const.tile([M, OUT_SH], mybir.dt.float16)
        nc.sync.dma_start(out=bs_sb[:], in_=bs.ap())

        QT = 4  # pair-tiles per ACT quad-copy

        for oc in range(N_OC):
            wt = wpool.tile([128, NT, OC], mybir.dt.uint16, tag="wt")
            nc.sync.dma_start(
                out=wt[:], in_=w16.ap()[oc * OC:(oc + 1) * OC, :],
                transpose=True)
            wt_b = wt[:].bitcast(mybir.dt.int8).rearrange(
                "p t (j e) -> p t j e", e=2)

            y_ps = pmain.tile([M, OC], mybir.dt.float32, tag="ymm")
            first = True
            for t0 in range(0, NT, QT):
                # ACT: one quad copy covering e=0 of t0..t0+3 (amortizes
                # its 224-cycle fixed overhead); DVE: four singles (e=1).
                eoq = eopool.tile([128, QT, OC], mybir.dt.float16, tag="eoq")
                nc.scalar.copy(eoq[:], wt_b[:, t0:t0 + QT, :, 0])
                for i in range(QT):
                    s = 2 * (t0 + i)
                    nc.tensor.matmul(
                        y_ps[:], xtp_sb[:, s, :], eoq[:, i, :],
                        start=first, stop=False)
                    first = False
                for i in range(QT):
                    s = 2 * (t0 + i) + 1
                    eo = eopool.tile([128, OC], mybir.dt.float16, tag="eo")
                    nc.vector.tensor_copy(eo[:], wt_b[:, t0 + i, :, 1])
                    nc.tensor.matmul(
                        y_ps[:], xtp_sb[:, s, :], eo[:],
                        start=False, stop=(s == NS - 1))

            o_ps = pout.tile([M, OC], mybir.dt.float32, tag="omm")
            nc.tensor.matmul(
                o_ps[:], aot_sb[:], wct_sb[:, oc * OC:(oc + 1) * OC],
                start=True, stop=True)

            # epilogue mirrors the reference's fp16 chain exactly,
            # including the overflowing int32->fp16 cast of y_int
            # (values > 65504 must become +/-inf):
            #   y16 = fp16(y_int); t1 = y16*xs; t2 = t1*sc;
            #   t3 = t2 + fp16(outlierGEMM); y = t3 + bias
            y16 = epool.tile([M, OC], mybir.dt.float16, tag="y16")
            nc.vector.tensor_copy(y16[:], y_ps[:])
            o16 = epool.tile([M, OC], mybir.dt.float16, tag="o16")
            nc.vector.tensor_copy(o16[:], o_ps[:])
            t1 = epool.tile([M, OC], mybir.dt.float16, tag="t1")
            nc.vector.tensor_scalar_mul(t1[:], y16[:], xs_sb[:])
            t2 = epool.tile([M, OC], mybir.dt.float16, tag="t2")
            nc.vector.tensor_mul(t2[:], t1[:], sc_sb[:, oc * OC:(oc + 1) * OC])
            t3 = epool.tile([M, OC], mybir.dt.float16, tag="t3")
            nc.vector.tensor_add(t3[:], t2[:], o16[:])
            yo = epool.tile([M, OC], mybir.dt.float16, tag="yo")
            nc.vector.tensor_add(yo[:], t3[:], bs_sb[:, oc * OC:(oc + 1) * OC])
            nc.sync.dma_start(out=y.ap()[:, oc * OC:(oc + 1) * OC], in_=yo[:])

    nc.compile()
    return nc


_NC_CACHE = None


def kernel(x, weight, scale_col, weight_cache, bias, ind):
    global LAST_RESULTS, _NC_CACHE

    x2 = np.asarray(x, dtype=np.float16).reshape(M, IN)
    weight = np.asarray(weight, dtype=np.int8)
    scale_col = np.asarray(scale_col, dtype=np.float16).reshape(OUT)
    weight_cache = np.asarray(weight_cache, dtype=np.float16)
    bias = np.asarray(bias, dtype=np.float16).reshape(OUT)
    ind = np.asarray(ind, dtype=np.int32)

    # ---- x-side prep (fp16 semantics to match reference) ----
    act_outliers = x2[:, ind]                              # [M, FP]
    tmp = x2.copy()
    tmp[:, ind] = np.float16(0)
    x_scale = np.max(np.abs(tmp), axis=1, keepdims=True) / np.float16(127)
    q = np.clip(np.round(tmp / x_scale), -128, 127).astype(np.float16)

    # xtp[k, 2t+e, m] = q[m, 256t + 2k + e]
    arr = q.reshape(M, NT, 128, 2)                         # [m, t, k, e]
    xtp = np.ascontiguousarray(arr.transpose(2, 1, 3, 0).reshape(128, NS, M))
    aot = np.ascontiguousarray(act_outliers.T)             # [FP, M]
    xs = x_scale.astype(np.float16).astype(np.float32)     # [M, 1], fp16-valued

    if _NC_CACHE is None:
        _NC_CACHE = _build_nc()
    nc = _NC_CACHE

    in_maps = []
    for c in range(N_CORES):
        lo, hi = c * OUT_SH, (c + 1) * OUT_SH
        w_sh = np.ascontiguousarray(weight[lo:hi]).view(np.uint16)
        in_maps.append({
            "w16": w_sh,
            "xtp": xtp,
            "aot": aot,
            "wct": np.ascontiguousarray(weight_cache[lo:hi].T),
            "xs": xs,
            "sc": np.ascontiguousarray(
                np.broadcast_to(scale_col[lo:hi], (M, OUT_SH))),
            "bs": np.ascontiguousarray(
                np.broadcast_to(bias[lo:hi], (M, OUT_SH))),
        })

    last_err = None
    for attempt in range(3):
        try:
            LAST_RESULTS = run_bass_kernel_spmd(
                nc, in_maps, list(range(N_CORES)))
            break
        except Exception as err:  # transient NRT exec-unit errors recover on retry
            last_err = err
            print(f"kernel: run attempt {attempt} failed ({type(err).__name__}); retrying",
                  file=sys.stderr)
            time.sleep(2.0)
    else:
        raise last_err
    parts = [LAST_RESULTS.results[c]["y"] for c in range(N_CORES)]
    out = np.concatenate(parts, axis=1).reshape(M, 1, OUT)
    return out.astype(np.float16)
